# revision 11
# baseline (speedup 1.0000x reference)
"""Trainium2 Bass kernel for BeliefPlausibility (Dempster-Shafer bel/pl maps).

Problem: input [4, 384, 1248, 7] fp32 (6 singleton masses + omega per pixel).
Output: tuple (bel, pl), each [4, 384, 1248, 64] fp32 where, per pixel with
masses m_0..m_5 and omega w:
    bel[q] = sum_c m_c * ((q >> c) & 1)  for q in 1..62;  bel[0]=0, bel[63]=1
    pl[q]  = bel[q] + w                  for q in 1..62;  pl[0]=0,  pl[63]=1

Strategy (pure data parallel over 8 cores, no cross-device communication):
  - The kernel is memory-bound: outputs are 2 x 64 channels vs 7 input
    channels.  Everything runs in bf16 (inputs host-cast, outputs
    host-upcast); the 2e-2 relative-error budget dwarfs bf16's 2^-9
    rounding, and halving the output bytes halves the HBM-write floor.
    The four constant output columns (bel/pl 0 and 63) are never moved:
    the device writes packed 62-column tensors and the host fills the
    constants while upcasting.
  - Each core gets 239,616 pixels.  The host pre-permutes its shard to
    lhsT layout [112, 117*128]: row 7j+c = channel c of pixel-group j,
    column t*128+blk = pixel block.  The whole shard (30 KB/partition)
    is DMA'd into SBUF once (in 8 chunks so compute starts early) and
    sliced per supertile -- no PE transpose, no per-tile input DMA.
  - Per supertile t (117 of them, 2048 pixels each): two bf16 matmuls
    [112,128] x [112,512] -> one PSUM bank pair [128, 1024] give bel
    for 16 pixel groups x 64 subsets, accumulated exactly in fp32.  The
    weight matrix also routes omega into column 63 of each group.  One
    ACT copy casts bel columns 1..62 PSUM->SBUF bf16 (packed); one DVE
    add forms pl = bel + omega (omega broadcast straight from PSUM
    column 63 with a zero-stride AP) writing packed bf16.
  - bel/pl SBUF staging is 4 persistent buffers x 3 supertiles; each
    buffer drains with one contiguous ~744 KB DMA per output tensor
    (the last two groups drain per-tile to shorten the pipeline tail),
    keeping the loop at ~5 instructions/tile.
"""

import sys

if "concourse" not in sys.modules:
    try:
        import concourse  # noqa: F401
    except ImportError:
        sys.path.insert(0, "/opt/trn_rl_repo")

import ml_dtypes
import numpy as np

import concourse.bacc as bacc
import concourse.bass as bass
import concourse.mybir as mybir
import concourse.tile as tile
from concourse.bass_utils import run_bass_kernel_spmd

F32 = mybir.dt.float32
BF16 = mybir.dt.bfloat16

N_CORES = 8
PX_TOTAL = 4 * 384 * 1248          # 1,916,928 pixels
PX_CORE = PX_TOTAL // N_CORES      # 239,616
PX_PART = 16                       # pixel groups per block (partition)
PX_TILE = 128 * PX_PART            # 2048 pixels per supertile
N_TILES = PX_CORE // PX_TILE       # 117
N_CH = 7                           # 6 singletons + omega
N_SUB = 64                         # output positions per pixel
N_PK = N_SUB - 2                   # 62 non-constant outputs per pixel
K_ROWS = PX_PART * N_CH            # 112 contraction rows
MM_W = PX_PART * N_SUB             # 1024 PSUM columns per tile
PK_W = PX_PART * N_PK              # 992 packed outputs per partition/tile
N_PS = 4                           # PSUM bank-pair rotation depth
OUT_GRP = 3                        # supertiles per output staging buffer
N_OBUF = 4                         # output staging buffers (bel & pl each)
TAIL_TILES = 6                     # final tiles drained per-tile


def _weight_matrix() -> np.ndarray:
    """[112, 1024]: W[7j+c, 64j+q] = (q>>c)&1 for q in 1..62, c in 0..5;
    W[7j+6, 64j+63] = 1 (omega lane for the pl broadcast)."""
    w = np.zeros((K_ROWS, MM_W), np.float32)
    for j in range(PX_PART):
        for q in range(1, 63):
            for c in range(6):
                if (q >> c) & 1:
                    w[7 * j + c, 64 * j + q] = 1.0
        w[7 * j + 6, 64 * j + 63] = 1.0
    return w


def build_program(n_tiles: int = N_TILES, reps: int = 1,
                  out_grp: int = OUT_GRP) -> bass.Bass:
    # Bacc (not plain Bass): its compile() runs generate_event_semaphores,
    # which splits multi-semaphore waits into standalone event-sem
    # instructions (TRN2 allows at most one wait per instruction).
    assert n_tiles % out_grp == 0
    nc = bacc.Bacc("TRN2")

    x = nc.dram_tensor("x", (K_ROWS, n_tiles * 128), BF16,
                       kind="ExternalInput")
    bel = nc.dram_tensor("bel", (n_tiles, 128, PK_W), BF16,
                         kind="ExternalOutput")
    pl = nc.dram_tensor("pl", (n_tiles, 128, PK_W), BF16,
                        kind="ExternalOutput")

    w_dram = nc.inline_tensor(
        _weight_matrix().astype(ml_dtypes.bfloat16), name="wmat")

    with tile.TileContext(nc) as tc:
        with (
            tc.tile_pool(name="const", bufs=1) as cpool,
            tc.tile_pool(name="outb", bufs=1) as belpool,
            tc.tile_pool(name="outp", bufs=1) as plpool,
            tc.tile_pool(name="psM", bufs=1, space="PSUM") as psMpool,
        ):
            wmat = cpool.tile([K_ROWS, MM_W], BF16)
            nc.sync.dma_start(wmat[:], w_dram[:])
            # Chunked input prefetch: the tile framework tracks byte-range
            # deps, so matmul t only waits for its own chunk and compute
            # starts ~1 chunk into the load instead of after all 3.35 MB.
            x_all = cpool.tile([K_ROWS, n_tiles * 128], BF16)
            n_ch_dma = 8
            ct = (n_tiles + n_ch_dma - 1) // n_ch_dma
            for k in range(0, n_tiles, ct):
                cols = slice(k * 128, min(n_tiles, k + ct) * 128)
                nc.sync.dma_start(x_all[:, cols], x[:, cols])

            # Persistent slot-cycled tensors: PSUM bank pairs for the
            # matmuls, and bel/pl staging buffers of OUT_GRP supertiles.
            ps_all = psMpool.tile([128, N_PS * MM_W], F32)
            gw = out_grp * PK_W
            bel_all = belpool.tile([128, N_OBUF * gw], BF16)
            pl_all = plpool.tile([128, N_OBUF * gw], BF16)

            for it in range(reps * n_tiles):
                t = it % n_tiles
                grp, tt = divmod(t, out_grp)
                buf = grp % N_OBUF
                ps = ps_all[:, MM_W * (it % N_PS):MM_W * (it % N_PS + 1)]
                ps3 = ps.rearrange("p (g q) -> p g q", q=N_SUB)
                lhsT = x_all[:, t * 128:(t + 1) * 128]
                off = buf * gw + tt * PK_W
                bel3 = bel_all[:, off:off + PK_W].rearrange(
                    "p (g q) -> p g q", q=N_PK)
                pl3 = pl_all[:, off:off + PK_W].rearrange(
                    "p (g q) -> p g q", q=N_PK)

                for h in range(2):
                    nc.tensor.matmul(ps[:, 512 * h:512 * (h + 1)], lhsT,
                                     wmat[:, 512 * h:512 * (h + 1)])

                # bel columns 1..62 of each group: ACT casts PSUM->bf16
                nc.scalar.copy(bel3[:, :, 0:N_PK], ps3[:, :, 1:63])

                # pl cols 1..62: bel + omega, omega broadcast straight
                # from PSUM column 63 via a zero-stride AP
                om = ps3[:, :, 63:64]
                om = bass.AP(om.tensor, om.offset, om.ap[:-1] + [[0, N_PK]])
                nc.vector.tensor_add(pl3[:, :, 0:N_PK],
                                     bel3[:, :, 0:N_PK], om)

                last = (t >= n_tiles - TAIL_TILES) and \
                    reps * n_tiles - it <= TAIL_TILES
                if last:
                    # Tail drains per-tile so the final DMAs start as soon
                    # as each tile's data is ready.
                    nc.sync.dma_start(bel[t], bel_all[:, off:off + PK_W])
                    nc.sync.dma_start(pl[t], pl_all[:, off:off + PK_W])
                elif tt == out_grp - 1:
                    # SBUF src stays partition-major; the DRAM dest AP is
                    # permuted to match its traversal order.
                    src_b = bel_all[:, buf * gw:(buf + 1) * gw].rearrange(
                        "p (s w) -> p s w", w=PK_W)
                    src_p = pl_all[:, buf * gw:(buf + 1) * gw].rearrange(
                        "p (s w) -> p s w", w=PK_W)
                    dst_b = bel[grp * out_grp:(grp + 1) * out_grp].rearrange(
                        "s p w -> p s w")
                    dst_p = pl[grp * out_grp:(grp + 1) * out_grp].rearrange(
                        "s p w -> p s w")
                    nc.sync.dma_start(dst_b, src_b)
                    nc.sync.dma_start(dst_p, src_p)

    nc.compile()
    return nc


_NC_CACHE: dict[int, bass.Bass] = {}


def _get_program(n_tiles: int) -> bass.Bass:
    if n_tiles not in _NC_CACHE:
        _NC_CACHE[n_tiles] = build_program(n_tiles)
    return _NC_CACHE[n_tiles]


def run_on_cores(x_flat: np.ndarray, **run_kwargs):
    """x_flat: [PX_TOTAL, 7] fp32. Returns (bel, pl) each [PX_TOTAL, 64]
    fp32, plus the raw BassKernelResults as third element."""
    nc = _get_program(N_TILES)
    in_maps = []
    for c in range(N_CORES):
        seg = x_flat[c * PX_CORE:(c + 1) * PX_CORE]
        # [t, blk, j, c] -> rows (j, c), cols (t, blk): lhsT layout
        x4 = seg.reshape(N_TILES, 128, PX_PART, N_CH)
        xp = x4.transpose(2, 3, 0, 1).reshape(K_ROWS, N_TILES * 128)
        in_maps.append({"x": np.ascontiguousarray(
            xp.astype(ml_dtypes.bfloat16))})
    rr = run_bass_kernel_spmd(nc, in_maps, core_ids=list(range(N_CORES)),
                              **run_kwargs)
    bel = np.empty((PX_TOTAL, N_SUB), np.float32)
    pl = np.empty((PX_TOTAL, N_SUB), np.float32)
    # constant columns never leave the device
    for arr in (bel, pl):
        arr[:, 0] = 0.0
        arr[:, 63] = 1.0
    for c, res in enumerate(rr.results):
        sl = slice(c * PX_CORE, (c + 1) * PX_CORE)
        bel[sl, 1:63] = np.asarray(res["bel"]).reshape(PX_CORE, N_PK)
        pl[sl, 1:63] = np.asarray(res["pl"]).reshape(PX_CORE, N_PK)
    return bel, pl, rr


def kernel(inputs: np.ndarray):
    inputs = np.ascontiguousarray(np.asarray(inputs, dtype=np.float32))
    b, hh, ww, ch = inputs.shape
    x_flat = inputs.reshape(-1, ch)
    bel, pl, _ = run_on_cores(x_flat)
    return (bel.reshape(b, hh, ww, N_SUB), pl.reshape(b, hh, ww, N_SUB))


# revision 17
# speedup vs baseline: 1.0061x; 1.0061x over previous
"""Trainium2 Bass kernel for BeliefPlausibility (Dempster-Shafer bel/pl maps).

Problem: input [4, 384, 1248, 7] fp32 (6 singleton masses + omega per pixel).
Output: tuple (bel, pl), each [4, 384, 1248, 64] fp32 where, per pixel with
masses m_0..m_5 and omega w:
    bel[q] = sum_c m_c * ((q >> c) & 1)  for q in 1..62;  bel[0]=0, bel[63]=1
    pl[q]  = bel[q] + w                  for q in 1..62;  pl[0]=0,  pl[63]=1

Strategy (pure data parallel over 8 cores, no cross-device communication):
  - The kernel is memory-bound: outputs are 2 x 64 channels vs 7 input
    channels.  Everything runs in bf16 (inputs host-cast, outputs
    host-upcast); the 2e-2 relative-error budget dwarfs bf16's 2^-9
    rounding, and halving the output bytes halves the HBM-write floor.
    (A packed layout skipping the 4 constant output columns was ~3 us
    slower: 1984 B descriptors lose more to alignment than the bytes
    save.  PACKED=False keeps 2 KB-aligned rows.)
  - Each core gets 239,616 pixels.  The host pre-permutes its shard to
    lhsT layout [112, 117*128]: row 7j+c = channel c of pixel-group j,
    column t*128+blk = pixel block.  The whole shard (30 KB/partition)
    is DMA'd into SBUF once (in 8 chunks so compute starts early) and
    sliced per supertile -- no PE transpose, no per-tile input DMA.
  - Per supertile t (117 of them, 2048 pixels each): two bf16 matmuls
    [112,128] x [112,512] -> one PSUM bank pair [128, 1024] give bel
    for 16 pixel groups x 64 subsets, accumulated exactly in fp32.  The
    weight matrix also routes omega into column 63 of each group.  One
    ACT copy casts bel columns 0..62 PSUM->SBUF bf16; one DVE add forms
    pl = bel + omega (omega broadcast straight from PSUM column 63 with
    a zero-stride AP).  Constant columns (bel/pl 63, pl 0) are written
    once per staging buffer, off the per-tile path.
  - bel/pl SBUF staging is 4 persistent buffers x 3 supertiles; each
    buffer drains with one contiguous ~744 KB DMA per output tensor
    (the last two groups drain per-tile to shorten the pipeline tail),
    keeping the loop at ~5 instructions/tile.
"""

import sys

if "concourse" not in sys.modules:
    try:
        import concourse  # noqa: F401
    except ImportError:
        sys.path.insert(0, "/opt/trn_rl_repo")

import ml_dtypes
import numpy as np

import concourse.bacc as bacc
import concourse.bass as bass
import concourse.mybir as mybir
import concourse.tile as tile
from concourse.bass_utils import run_bass_kernel_spmd

F32 = mybir.dt.float32
BF16 = mybir.dt.bfloat16

N_CORES = 8
PX_TOTAL = 4 * 384 * 1248          # 1,916,928 pixels
PX_CORE = PX_TOTAL // N_CORES      # 239,616
PX_PART = 16                       # pixel groups per block (partition)
PX_TILE = 128 * PX_PART            # 2048 pixels per supertile
N_TILES = PX_CORE // PX_TILE       # 117
N_CH = 7                           # 6 singletons + omega
N_SUB = 64                         # output positions per pixel
N_PK = N_SUB - 2                   # 62 non-constant outputs per pixel
K_ROWS = PX_PART * N_CH            # 112 contraction rows
MM_W = PX_PART * N_SUB             # 1024 PSUM columns per tile
PK_W = PX_PART * N_PK              # 992 packed outputs per partition/tile
N_PS = 4                           # PSUM bank-pair rotation depth
OUT_GRP = 3                        # supertiles per output staging buffer
N_OBUF = 4                         # output staging buffers (bel & pl each)
TAIL_TILES = 6                     # final tiles drained per-tile
HEAD_TILES = 0                     # initial tiles drained per-tile
CHUNKS = [15] * 8                  # input prefetch chunk sizes (tiles)
PACKED = False                     # skip constant output columns on device


def _weight_matrix() -> np.ndarray:
    """[112, 1024]: W[7j+c, 64j+q] = (q>>c)&1 for q in 1..62, c in 0..5;
    W[7j+6, 64j+63] = 1 (omega lane for the pl broadcast)."""
    w = np.zeros((K_ROWS, MM_W), np.float32)
    for j in range(PX_PART):
        for q in range(1, 63):
            for c in range(6):
                if (q >> c) & 1:
                    w[7 * j + c, 64 * j + q] = 1.0
        w[7 * j + 6, 64 * j + 63] = 1.0
    return w


def build_program(n_tiles: int = N_TILES, reps: int = 1,
                  out_grp: int = OUT_GRP,
                  packed: bool | None = None) -> bass.Bass:
    # Bacc (not plain Bass): its compile() runs generate_event_semaphores,
    # which splits multi-semaphore waits into standalone event-sem
    # instructions (TRN2 allows at most one wait per instruction).
    assert n_tiles % out_grp == 0
    if packed is None:
        packed = PACKED
    out_w = PK_W if packed else MM_W
    nc = bacc.Bacc("TRN2")

    x = nc.dram_tensor("x", (K_ROWS, n_tiles * 128), BF16,
                       kind="ExternalInput")
    bel = nc.dram_tensor("bel", (n_tiles, 128, out_w), BF16,
                         kind="ExternalOutput")
    pl = nc.dram_tensor("pl", (n_tiles, 128, out_w), BF16,
                        kind="ExternalOutput")

    w_dram = nc.inline_tensor(
        _weight_matrix().astype(ml_dtypes.bfloat16), name="wmat")

    with tile.TileContext(nc) as tc:
        with (
            tc.tile_pool(name="const", bufs=1) as cpool,
            tc.tile_pool(name="outb", bufs=1) as belpool,
            tc.tile_pool(name="outp", bufs=1) as plpool,
            tc.tile_pool(name="psM", bufs=1, space="PSUM") as psMpool,
        ):
            wmat = cpool.tile([K_ROWS, MM_W], BF16)
            nc.sync.dma_start(wmat[:], w_dram[:])
            # Chunked input prefetch: the tile framework tracks byte-range
            # deps, so matmul t only waits for its own chunk and compute
            # starts ~1 chunk into the load instead of after all 3.35 MB.
            x_all = cpool.tile([K_ROWS, n_tiles * 128], BF16)
            k = 0
            for ct in CHUNKS:
                if k >= n_tiles:
                    break
                cols = slice(k * 128, min(n_tiles, k + ct) * 128)
                nc.sync.dma_start(x_all[:, cols], x[:, cols])
                k += ct

            # Persistent slot-cycled tensors: PSUM bank pairs for the
            # matmuls, and bel/pl staging buffers of OUT_GRP supertiles.
            ps_all = psMpool.tile([128, N_PS * MM_W], F32)
            gw = out_grp * out_w
            bel_all = belpool.tile([128, N_OBUF * gw], BF16)
            pl_all = plpool.tile([128, N_OBUF * gw], BF16)
            if not packed:
                bel4 = bel_all[:].rearrange("p (b g q) -> p b g q",
                                            b=N_OBUF, q=N_SUB)
                pl4 = pl_all[:].rearrange("p (b g q) -> p b g q",
                                          b=N_OBUF, q=N_SUB)
                for s in range(N_OBUF):
                    nc.vector.memset(bel4[:, s, :, 63:64], 1.0)
                    nc.vector.memset(pl4[:, s, :, 0:1], 0.0)
                    nc.vector.memset(pl4[:, s, :, 63:64], 1.0)

            for it in range(reps * n_tiles):
                t = it % n_tiles
                grp, tt = divmod(t, out_grp)
                buf = grp % N_OBUF
                ps = ps_all[:, MM_W * (it % N_PS):MM_W * (it % N_PS + 1)]
                ps3 = ps.rearrange("p (g q) -> p g q", q=N_SUB)
                lhsT = x_all[:, t * 128:(t + 1) * 128]
                off = buf * gw + tt * out_w
                outq = N_PK if packed else N_SUB
                bel3 = bel_all[:, off:off + out_w].rearrange(
                    "p (g q) -> p g q", q=outq)
                pl3 = pl_all[:, off:off + out_w].rearrange(
                    "p (g q) -> p g q", q=outq)

                for h in range(2):
                    nc.tensor.matmul(ps[:, 512 * h:512 * (h + 1)], lhsT,
                                     wmat[:, 512 * h:512 * (h + 1)])

                # bel columns 1..62 of each group: ACT casts PSUM->bf16
                # (packed: dropped constant cols; else cols 0..62 w/ col 0
                # coming from the all-zero weight column)
                lo = 0 if packed else 1
                nc.scalar.copy(bel3[:, :, lo:lo + N_PK]
                               if packed else bel3[:, :, 0:63],
                               ps3[:, :, 1:63] if packed
                               else ps3[:, :, 0:63])

                # pl cols 1..62: bel + omega, omega broadcast straight
                # from PSUM column 63 via a zero-stride AP
                om = ps3[:, :, 63:64]
                om = bass.AP(om.tensor, om.offset, om.ap[:-1] + [[0, N_PK]])
                nc.vector.tensor_add(pl3[:, :, lo:lo + N_PK],
                                     bel3[:, :, lo:lo + N_PK], om)

                last = (t >= n_tiles - TAIL_TILES) and \
                    reps * n_tiles - it <= TAIL_TILES
                head = it < HEAD_TILES
                if last or head:
                    # Tail drains per-tile so the final DMAs start as soon
                    # as each tile's data is ready.
                    nc.sync.dma_start(bel[t], bel_all[:, off:off + out_w])
                    nc.sync.dma_start(pl[t], pl_all[:, off:off + out_w])
                elif tt == out_grp - 1:
                    # SBUF src stays partition-major; the DRAM dest AP is
                    # permuted to match its traversal order.
                    src_b = bel_all[:, buf * gw:(buf + 1) * gw].rearrange(
                        "p (s w) -> p s w", w=out_w)
                    src_p = pl_all[:, buf * gw:(buf + 1) * gw].rearrange(
                        "p (s w) -> p s w", w=out_w)
                    dst_b = bel[grp * out_grp:(grp + 1) * out_grp].rearrange(
                        "s p w -> p s w")
                    dst_p = pl[grp * out_grp:(grp + 1) * out_grp].rearrange(
                        "s p w -> p s w")
                    nc.sync.dma_start(dst_b, src_b)
                    nc.sync.dma_start(dst_p, src_p)

    nc.compile()
    return nc


_NC_CACHE: dict[int, bass.Bass] = {}


def _get_program(n_tiles: int) -> bass.Bass:
    if n_tiles not in _NC_CACHE:
        _NC_CACHE[n_tiles] = build_program(n_tiles)
    return _NC_CACHE[n_tiles]


def run_on_cores(x_flat: np.ndarray, **run_kwargs):
    """x_flat: [PX_TOTAL, 7] fp32. Returns (bel, pl) each [PX_TOTAL, 64]
    fp32, plus the raw BassKernelResults as third element."""
    nc = _get_program(N_TILES)
    in_maps = []
    for c in range(N_CORES):
        seg = x_flat[c * PX_CORE:(c + 1) * PX_CORE]
        # [t, blk, j, c] -> rows (j, c), cols (t, blk): lhsT layout
        x4 = seg.reshape(N_TILES, 128, PX_PART, N_CH)
        xp = x4.transpose(2, 3, 0, 1).reshape(K_ROWS, N_TILES * 128)
        in_maps.append({"x": np.ascontiguousarray(
            xp.astype(ml_dtypes.bfloat16))})
    rr = run_bass_kernel_spmd(nc, in_maps, core_ids=list(range(N_CORES)),
                              **run_kwargs)
    bel = np.empty((PX_TOTAL, N_SUB), np.float32)
    pl = np.empty((PX_TOTAL, N_SUB), np.float32)
    if PACKED:
        # constant columns never leave the device
        for arr in (bel, pl):
            arr[:, 0] = 0.0
            arr[:, 63] = 1.0
        for c, res in enumerate(rr.results):
            sl = slice(c * PX_CORE, (c + 1) * PX_CORE)
            bel[sl, 1:63] = np.asarray(res["bel"]).reshape(PX_CORE, N_PK)
            pl[sl, 1:63] = np.asarray(res["pl"]).reshape(PX_CORE, N_PK)
    else:
        for c, res in enumerate(rr.results):
            sl = slice(c * PX_CORE, (c + 1) * PX_CORE)
            bel[sl] = np.asarray(res["bel"]).reshape(PX_CORE, N_SUB)
            pl[sl] = np.asarray(res["pl"]).reshape(PX_CORE, N_SUB)
    return bel, pl, rr


def kernel(inputs: np.ndarray):
    inputs = np.ascontiguousarray(np.asarray(inputs, dtype=np.float32))
    b, hh, ww, ch = inputs.shape
    x_flat = inputs.reshape(-1, ch)
    bel, pl, _ = run_on_cores(x_flat)
    return (bel.reshape(b, hh, ww, N_SUB), pl.reshape(b, hh, ww, N_SUB))


# revision 18
# speedup vs baseline: 1.0130x; 1.0069x over previous
"""Trainium2 Bass kernel for BeliefPlausibility (Dempster-Shafer bel/pl maps).

Problem: input [4, 384, 1248, 7] fp32 (6 singleton masses + omega per pixel).
Output: tuple (bel, pl), each [4, 384, 1248, 64] fp32 where, per pixel with
masses m_0..m_5 and omega w:
    bel[q] = sum_c m_c * ((q >> c) & 1)  for q in 1..62;  bel[0]=0, bel[63]=1
    pl[q]  = bel[q] + w                  for q in 1..62;  pl[0]=0,  pl[63]=1

Strategy (pure data parallel over 8 cores, no cross-device communication):
  - The kernel is memory-bound: outputs are 2 x 64 channels vs 7 input
    channels.  Everything runs in bf16 (inputs host-cast, outputs
    host-upcast); the 2e-2 relative-error budget dwarfs bf16's 2^-9
    rounding, and halving the output bytes halves the HBM-write floor.
    (A packed layout skipping the 4 constant output columns was ~3 us
    slower: 1984 B descriptors lose more to alignment than the bytes
    save.  PACKED=False keeps 2 KB-aligned rows.)
  - Each core gets 239,616 pixels.  The host pre-permutes its shard to
    lhsT layout [112, 117*128]: row 7j+c = channel c of pixel-group j,
    column t*128+blk = pixel block.  The whole shard (30 KB/partition)
    is DMA'd into SBUF once (in 8 chunks so compute starts early) and
    sliced per supertile -- no PE transpose, no per-tile input DMA.
  - Per supertile t (117 of them, 2048 pixels each): two bf16 matmuls
    [112,128] x [112,512] -> one PSUM bank pair [128, 1024] give bel
    for 16 pixel groups x 64 subsets, accumulated exactly in fp32.  The
    weight matrix also routes omega into column 63 of each group.  One
    ACT copy casts bel columns 0..62 PSUM->SBUF bf16; one DVE add forms
    pl = bel + omega (omega broadcast straight from PSUM column 63 with
    a zero-stride AP).  Constant columns (bel/pl 63, pl 0) are written
    once per staging buffer, off the per-tile path.
  - bel/pl SBUF staging is 4 persistent buffers x 3 supertiles; each
    buffer drains with one contiguous ~744 KB DMA per output tensor
    (the last two groups drain per-tile to shorten the pipeline tail),
    keeping the loop at ~5 instructions/tile.
"""

import sys

if "concourse" not in sys.modules:
    try:
        import concourse  # noqa: F401
    except ImportError:
        sys.path.insert(0, "/opt/trn_rl_repo")

import ml_dtypes
import numpy as np

import concourse.bacc as bacc
import concourse.bass as bass
import concourse.mybir as mybir
import concourse.tile as tile
from concourse.bass_utils import run_bass_kernel_spmd

F32 = mybir.dt.float32
BF16 = mybir.dt.bfloat16

N_CORES = 8
PX_TOTAL = 4 * 384 * 1248          # 1,916,928 pixels
PX_CORE = PX_TOTAL // N_CORES      # 239,616
PX_PART = 16                       # pixel groups per block (partition)
PX_TILE = 128 * PX_PART            # 2048 pixels per supertile
N_TILES = PX_CORE // PX_TILE       # 117
N_CH = 7                           # 6 singletons + omega
N_SUB = 64                         # output positions per pixel
N_PK = N_SUB - 2                   # 62 non-constant outputs per pixel
K_ROWS = PX_PART * N_CH            # 112 contraction rows
MM_W = PX_PART * N_SUB             # 1024 PSUM columns per tile
PK_W = PX_PART * N_PK              # 992 packed outputs per partition/tile
N_PS = 4                           # PSUM bank-pair rotation depth
OUT_GRP = 3                        # supertiles per output staging buffer
N_OBUF = 4                         # output staging buffers (bel & pl each)
TAIL_TILES = 6                     # final tiles drained per-tile
HEAD_TILES = 0                     # initial tiles drained per-tile
CHUNKS = [15] * 8                  # input prefetch chunk sizes (tiles)
PACKED = False                     # skip constant output columns on device


def _weight_matrix() -> np.ndarray:
    """[112, 1024]: W[7j+c, 64j+q] = (q>>c)&1 for q in 1..62, c in 0..5;
    W[7j+6, 64j+63] = 1 (omega lane for the pl broadcast)."""
    w = np.zeros((K_ROWS, MM_W), np.float32)
    for j in range(PX_PART):
        for q in range(1, 63):
            for c in range(6):
                if (q >> c) & 1:
                    w[7 * j + c, 64 * j + q] = 1.0
        w[7 * j + 6, 64 * j + 63] = 1.0
    return w


def build_program(n_tiles: int = N_TILES, reps: int = 1,
                  out_grp: int = OUT_GRP,
                  packed: bool | None = None) -> bass.Bass:
    # Bacc (not plain Bass): its compile() runs generate_event_semaphores,
    # which splits multi-semaphore waits into standalone event-sem
    # instructions (TRN2 allows at most one wait per instruction).
    assert n_tiles % out_grp == 0
    if packed is None:
        packed = PACKED
    out_w = PK_W if packed else MM_W
    nc = bacc.Bacc("TRN2")

    x = nc.dram_tensor("x", (K_ROWS, n_tiles * 128), BF16,
                       kind="ExternalInput")
    bel = nc.dram_tensor("bel", (n_tiles, 128, out_w), BF16,
                         kind="ExternalOutput")
    pl = nc.dram_tensor("pl", (n_tiles, 128, out_w), BF16,
                        kind="ExternalOutput")

    w_dram = nc.inline_tensor(
        _weight_matrix().astype(ml_dtypes.bfloat16), name="wmat")

    with tile.TileContext(nc) as tc:
        with (
            tc.tile_pool(name="const", bufs=1) as cpool,
            tc.tile_pool(name="outb", bufs=1) as belpool,
            tc.tile_pool(name="outp", bufs=1) as plpool,
            tc.tile_pool(name="psM", bufs=1, space="PSUM") as psMpool,
        ):
            wmat = cpool.tile([K_ROWS, MM_W], BF16)
            nc.sync.dma_start(wmat[:], w_dram[:])
            # Chunked input prefetch: the tile framework tracks byte-range
            # deps, so matmul t only waits for its own chunk and compute
            # starts ~1 chunk into the load instead of after all 3.35 MB.
            x_all = cpool.tile([K_ROWS, n_tiles * 128], BF16)
            k = 0
            for ct in CHUNKS:
                if k >= n_tiles:
                    break
                cols = slice(k * 128, min(n_tiles, k + ct) * 128)
                nc.sync.dma_start(x_all[:, cols], x[:, cols])
                k += ct

            # Persistent slot-cycled tensors: PSUM bank pairs for the
            # matmuls, and bel/pl staging buffers of OUT_GRP supertiles.
            ps_all = psMpool.tile([128, N_PS * MM_W], F32)
            gw = out_grp * out_w
            bel_all = belpool.tile([128, N_OBUF * gw], BF16)
            pl_all = plpool.tile([128, N_OBUF * gw], BF16)
            if not packed:
                bel4 = bel_all[:].rearrange("p (b g q) -> p b g q",
                                            b=N_OBUF, q=N_SUB)
                pl4 = pl_all[:].rearrange("p (b g q) -> p b g q",
                                          b=N_OBUF, q=N_SUB)
                for s in range(N_OBUF):
                    nc.vector.memset(bel4[:, s, :, 63:64], 1.0)
                    nc.vector.memset(pl4[:, s, :, 0:1], 0.0)
                    nc.vector.memset(pl4[:, s, :, 63:64], 1.0)

            for it in range(reps * n_tiles):
                t = it % n_tiles
                grp, tt = divmod(t, out_grp)
                buf = grp % N_OBUF
                ps = ps_all[:, MM_W * (it % N_PS):MM_W * (it % N_PS + 1)]
                ps3 = ps.rearrange("p (g q) -> p g q", q=N_SUB)
                lhsT = x_all[:, t * 128:(t + 1) * 128]
                off = buf * gw + tt * out_w
                outq = N_PK if packed else N_SUB
                bel3 = bel_all[:, off:off + out_w].rearrange(
                    "p (g q) -> p g q", q=outq)
                pl3 = pl_all[:, off:off + out_w].rearrange(
                    "p (g q) -> p g q", q=outq)

                for h in range(2):
                    nc.tensor.matmul(ps[:, 512 * h:512 * (h + 1)], lhsT,
                                     wmat[:, 512 * h:512 * (h + 1)])

                # bel columns 1..62 of each group: ACT casts PSUM->bf16
                # (packed: dropped constant cols; else cols 0..62 w/ col 0
                # coming from the all-zero weight column)
                lo = 0 if packed else 1
                nc.scalar.copy(bel3[:, :, lo:lo + N_PK]
                               if packed else bel3[:, :, 0:63],
                               ps3[:, :, 1:63] if packed
                               else ps3[:, :, 0:63])

                # pl cols 1..62: bel + omega, omega broadcast straight
                # from PSUM column 63 via a zero-stride AP
                om = ps3[:, :, 63:64]
                om = bass.AP(om.tensor, om.offset, om.ap[:-1] + [[0, N_PK]])
                nc.vector.tensor_add(pl3[:, :, lo:lo + N_PK],
                                     bel3[:, :, lo:lo + N_PK], om)

                last = (t >= n_tiles - TAIL_TILES) and \
                    reps * n_tiles - it <= TAIL_TILES
                head = it < HEAD_TILES
                if last or head:
                    # Tail drains per-tile so the final DMAs start as soon
                    # as each tile's data is ready.
                    nc.sync.dma_start(bel[t], bel_all[:, off:off + out_w])
                    # pl drains on the scalar engine's HWDGE ring so the
                    # two physical rings split the output descriptor load
                    nc.scalar.dma_start(pl[t], pl_all[:, off:off + out_w])
                elif tt == out_grp - 1:
                    # SBUF src stays partition-major; the DRAM dest AP is
                    # permuted to match its traversal order.
                    src_b = bel_all[:, buf * gw:(buf + 1) * gw].rearrange(
                        "p (s w) -> p s w", w=out_w)
                    src_p = pl_all[:, buf * gw:(buf + 1) * gw].rearrange(
                        "p (s w) -> p s w", w=out_w)
                    dst_b = bel[grp * out_grp:(grp + 1) * out_grp].rearrange(
                        "s p w -> p s w")
                    dst_p = pl[grp * out_grp:(grp + 1) * out_grp].rearrange(
                        "s p w -> p s w")
                    nc.sync.dma_start(dst_b, src_b)
                    nc.scalar.dma_start(dst_p, src_p)

    nc.compile()
    return nc


_NC_CACHE: dict[int, bass.Bass] = {}


def _get_program(n_tiles: int) -> bass.Bass:
    if n_tiles not in _NC_CACHE:
        _NC_CACHE[n_tiles] = build_program(n_tiles)
    return _NC_CACHE[n_tiles]


def run_on_cores(x_flat: np.ndarray, **run_kwargs):
    """x_flat: [PX_TOTAL, 7] fp32. Returns (bel, pl) each [PX_TOTAL, 64]
    fp32, plus the raw BassKernelResults as third element."""
    nc = _get_program(N_TILES)
    in_maps = []
    for c in range(N_CORES):
        seg = x_flat[c * PX_CORE:(c + 1) * PX_CORE]
        # [t, blk, j, c] -> rows (j, c), cols (t, blk): lhsT layout
        x4 = seg.reshape(N_TILES, 128, PX_PART, N_CH)
        xp = x4.transpose(2, 3, 0, 1).reshape(K_ROWS, N_TILES * 128)
        in_maps.append({"x": np.ascontiguousarray(
            xp.astype(ml_dtypes.bfloat16))})
    rr = run_bass_kernel_spmd(nc, in_maps, core_ids=list(range(N_CORES)),
                              **run_kwargs)
    bel = np.empty((PX_TOTAL, N_SUB), np.float32)
    pl = np.empty((PX_TOTAL, N_SUB), np.float32)
    if PACKED:
        # constant columns never leave the device
        for arr in (bel, pl):
            arr[:, 0] = 0.0
            arr[:, 63] = 1.0
        for c, res in enumerate(rr.results):
            sl = slice(c * PX_CORE, (c + 1) * PX_CORE)
            bel[sl, 1:63] = np.asarray(res["bel"]).reshape(PX_CORE, N_PK)
            pl[sl, 1:63] = np.asarray(res["pl"]).reshape(PX_CORE, N_PK)
    else:
        for c, res in enumerate(rr.results):
            sl = slice(c * PX_CORE, (c + 1) * PX_CORE)
            bel[sl] = np.asarray(res["bel"]).reshape(PX_CORE, N_SUB)
            pl[sl] = np.asarray(res["pl"]).reshape(PX_CORE, N_SUB)
    return bel, pl, rr


def kernel(inputs: np.ndarray):
    inputs = np.ascontiguousarray(np.asarray(inputs, dtype=np.float32))
    b, hh, ww, ch = inputs.shape
    x_flat = inputs.reshape(-1, ch)
    bel, pl, _ = run_on_cores(x_flat)
    return (bel.reshape(b, hh, ww, N_SUB), pl.reshape(b, hh, ww, N_SUB))
